# revision 1
# baseline (speedup 1.0000x reference)
"""Trainium2 Bass kernel for a 2-layer bidirectional GRU decoder.

Problem (hardcoded shapes): B=32, T=1024, D=256, H=256, P=256.
  p1 = bidir_gru(decoder_input, h0, W1f/U1f, W1b/U1b)        # [B,T,512]
  p2 = bidir_gru(p1,            h0, W2f/U2f, W2b/U2b)        # [B,T,512]
  recon = tanh(p2[:, ::-1, :] @ Wp + bp)                     # [B,T,256]

Strategy: pure data parallel over batch (32 = 8 cores x 4). Each core runs
all four GRU scans for its batch shard; no cross-core traffic. Everything is
kept in a transposed layout [feature -> partitions, batch -> free] so that
the recurrent matmul streams h (tiny) against stationary U chunks (bf16,
fast-weight-load), gate pre-activations are injected into PSUM via an
identity matmul, and all per-feature biases fold into per-partition
activation bias operands.
"""

import sys

if "/opt/trn_rl_repo" not in sys.path:
    sys.path.insert(0, "/opt/trn_rl_repo")

import numpy as np
import ml_dtypes

import concourse.bass as bass
import concourse.bacc as bacc
import concourse.mybir as mybir
import concourse.tile as tile
from concourse.bass import ds, ts

BF16 = mybir.dt.bfloat16
F32 = mybir.dt.float32
AF = mybir.ActivationFunctionType
ALU = mybir.AluOpType

P = 128
D = 256
H = 256
H3 = 3 * H
KD = D // P            # 2  k-chunks of the layer-1 input dim
KH = H // P            # 2  k-chunks of the hidden dim
K2 = (2 * H) // P      # 4  k-chunks of the layer-2 input dim
M3 = H3 // P           # 6  m-chunks of the 3H gate dim
MP = 256 // P          # 2  m-chunks of the projection output
NCORES = 8
DIRS = ("f", "b")
DEFAULT_UNROLL = 8


def build_program(T: int, B: int, unroll: int, stop_after: int = 5):
    """Build the per-core SPMD program. Returns (nc, meta).

    stop_after: debug knob — 1=xp1, 2=+scan1, 3=+xp2, 4=+scan2, 5=full.
    """
    assert T % unroll == 0
    NT = T * B
    NCHU = min(512, NT)            # matmul moving-dim chunk
    assert NT % NCHU == 0
    NCHUNKS = NT // NCHU
    TCNT = NCHU // B               # timesteps covered per chunk

    nc = bacc.Bacc("TRN2", num_devices=NCORES, debug=False)

    # ---------------- DRAM I/O ----------------
    xT = nc.dram_tensor("xT", [KD, P, NT], BF16, kind="ExternalInput")
    h0T = nc.dram_tensor("h0T", [KH, P, B], F32, kind="ExternalInput")
    ident = nc.dram_tensor("ident", [P, P], F32, kind="ExternalInput")

    Wd, Ud, bzr, b0h, b1h = {}, {}, {}, {}, {}
    for li, kin in ((1, D), (2, 2 * H)):
        for d in DIRS:
            Wd[li, d] = nc.dram_tensor(f"W{li}{d}", [kin, H3], BF16, kind="ExternalInput")
            Ud[li, d] = nc.dram_tensor(f"U{li}{d}", [H, H3], BF16, kind="ExternalInput")
            bzr[li, d] = nc.dram_tensor(f"bzr{li}{d}", [2 * H], F32, kind="ExternalInput")
            b0h[li, d] = nc.dram_tensor(f"b0h{li}{d}", [H], F32, kind="ExternalInput")
            b1h[li, d] = nc.dram_tensor(f"b1h{li}{d}", [H], F32, kind="ExternalInput")
    Wp = nc.dram_tensor("Wp", [2 * H, 256], BF16, kind="ExternalInput")
    bp = nc.dram_tensor("bp", [256], F32, kind="ExternalInput")

    # scratch (per-core DRAM)
    xp_zr, xp_h = {}, {}
    for li in (1, 2):
        for d in DIRS:
            xp_zr[li, d] = nc.dram_tensor(f"xpzr{li}{d}", [T * P, 4 * B], F32, kind="Internal")
            xp_h[li, d] = nc.dram_tensor(f"xph{li}{d}", [T * P, 2 * B], F32, kind="Internal")
    p1T = nc.dram_tensor("p1T", [T * P, 4 * B], BF16, kind="Internal")
    p2T = nc.dram_tensor("p2T", [T * P, 4 * B], BF16, kind="Internal")

    reconT = nc.dram_tensor("reconT", [MP, P, T, B], F32, kind="ExternalOutput")

    with tile.TileContext(nc) as tc:
        with tc.tile_pool(name="persist", bufs=1) as pers:
            # ---- load persistent SBUF data ----
            xT_sb = pers.tile([P, KD, NT], BF16)
            for ko in range(KD):
                nc.sync.dma_start(xT_sb[:, ko, :], xT[ko])

            ident_sb = pers.tile([P, P], F32)
            nc.sync.dma_start(ident_sb, ident[:])

            h0_sb = pers.tile([P, KH, B], F32)
            nc.sync.dma_start(h0_sb, h0T.ap().rearrange("ko p b -> p ko b"))

            W_sb, U_sb, bzr_sb, b0h_sb, b1h_sb = {}, {}, {}, {}, {}
            for li, kin in ((1, D), (2, 2 * H)):
                for d in DIRS:
                    kck = kin // P
                    W_sb[li, d] = pers.tile([P, kck, M3, P], BF16, name=f"W{li}{d}_sb")
                    nc.sync.dma_start(
                        W_sb[li, d],
                        Wd[li, d].ap().rearrange("(ko p) (mc q) -> p ko mc q", p=P, q=P),
                    )
                    U_sb[li, d] = pers.tile([P, KH, M3, P], BF16, name=f"U{li}{d}_sb")
                    nc.sync.dma_start(
                        U_sb[li, d],
                        Ud[li, d].ap().rearrange("(ko p) (mc q) -> p ko mc q", p=P, q=P),
                    )
                    bzr_sb[li, d] = pers.tile([P, 4], F32, name=f"bzr{li}{d}_sb")
                    nc.sync.dma_start(
                        bzr_sb[li, d], bzr[li, d].ap().rearrange("(mc p) -> p mc", p=P)
                    )
                    b0h_sb[li, d] = pers.tile([P, 2], F32, name=f"b0h{li}{d}_sb")
                    nc.sync.dma_start(
                        b0h_sb[li, d], b0h[li, d].ap().rearrange("(mc p) -> p mc", p=P)
                    )
                    b1h_sb[li, d] = pers.tile([P, 2], F32, name=f"b1h{li}{d}_sb")
                    nc.sync.dma_start(
                        b1h_sb[li, d], b1h[li, d].ap().rearrange("(mc p) -> p mc", p=P)
                    )
            Wp_sb = pers.tile([P, K2, MP, P], BF16)
            nc.sync.dma_start(
                Wp_sb, Wp.ap().rearrange("(ko p) (mc q) -> p ko mc q", p=P, q=P)
            )
            bp_sb = pers.tile([P, MP], F32)
            nc.sync.dma_start(bp_sb, bp.ap().rearrange("(mc p) -> p mc", p=P))

            # GRU hidden state tiles (persist across one scan phase):
            # fp32 master (updated on gpsimd) + bf16 shadow fed to the PE
            h32 = {d: pers.tile([P, KH, B], F32, name=f"h32_{d}") for d in DIRS}
            hbf = {d: pers.tile([P, KH, B], BF16, name=f"hbf_{d}") for d in DIRS}

            # ============ input-projection phase ============
            def xp_phase(li: int, rhs_src, kck: int):
                """xp[li,d] = rhs @ W[li,d] + biases, written as per-step records.

                rhs_src(n_i, pool) -> SBUF AP [P, kck, NCHU] for columns of (t, b).
                """
                with (
                    tc.tile_pool(name=f"xpps{li}", bufs=4, space="PSUM") as psp,
                    tc.tile_pool(name=f"xpo{li}", bufs=4) as osp,
                    tc.tile_pool(name=f"xprhs{li}", bufs=3) as rhp,
                ):
                    for n_i in range(NCHUNKS):
                        rhs = rhs_src(n_i, rhp)
                        t0 = n_i * TCNT
                        for d in DIRS:
                            for mc in range(M3):
                                ps = psp.tile([P, NCHU], F32, tag="ps")
                                for ko in range(kck):
                                    nc.tensor.matmul(
                                        ps,
                                        W_sb[li, d][:, ko, mc, :],
                                        rhs[:, ko, :],
                                        start=(ko == 0),
                                        stop=(ko == kck - 1),
                                    )
                                if mc < 4:
                                    o = osp.tile([P, NCHU], F32, tag="ozr")
                                    dst = (
                                        xp_zr[li, d]
                                        .ap()
                                        .rearrange("(t p) (mc b) -> p mc t b", p=P, b=B)
                                    )[:, mc, ds(t0, TCNT), :]
                                else:
                                    o = osp.tile([P, NCHU], F32, tag="oh")
                                    dst = (
                                        xp_h[li, d]
                                        .ap()
                                        .rearrange("(t p) (mc b) -> p mc t b", p=P, b=B)
                                    )[:, mc - 4, ds(t0, TCNT), :]
                                if mc % 2 == 0:
                                    bias_ap = (
                                        bzr_sb[li, d][:, mc : mc + 1]
                                        if mc < 4
                                        else b0h_sb[li, d][:, mc - 4 : mc - 3]
                                    )
                                    nc.scalar.activation(o, ps, AF.Identity, bias=bias_ap)
                                else:
                                    bias_ap = (
                                        bzr_sb[li, d][:, mc : mc + 1]
                                        if mc < 4
                                        else b0h_sb[li, d][:, mc - 4 : mc - 3]
                                    )
                                    nc.vector.tensor_tensor(
                                        o, ps, bias_ap.to_broadcast((P, NCHU)), ALU.add
                                    )
                                nc.sync.dma_start(dst, o)

            # ============ recurrent scan phase ============
            def scan_phase(li: int, outT):
                # reset hidden state to h0
                for d in DIRS:
                    nc.vector.tensor_copy(h32[d], h0_sb)
                    nc.vector.tensor_copy(hbf[d], h0_sb)
                with (
                    tc.tile_pool(name=f"ld{li}", bufs=4) as ldp,
                    tc.tile_pool(name=f"ps{li}", bufs=2, space="PSUM") as psp,
                    tc.tile_pool(name=f"gt{li}", bufs=2) as gtp,
                ):
                    with tc.For_i(0, T, unroll) as i0:
                        for u in range(unroll):
                            for d in DIRS:
                                hT = hbf[d]
                                hm = h32[d]
                                if d == "f":
                                    trow = (i0 + u) * P
                                else:
                                    trow = ((T - 1 - u) - i0) * P
                                srow = (i0 + u) * P  # storage row (scan index)

                                xz = ldp.tile([P, 4, B], F32, tag=f"xz{d}")
                                nc.sync.dma_start(
                                    xz,
                                    xp_zr[li, d]
                                    .ap()
                                    .rearrange("r (mc b) -> r mc b", b=B)[ds(trow, P)],
                                )
                                xh = ldp.tile([P, 2, B], F32, tag=f"xh{d}")
                                nc.sync.dma_start(
                                    xh,
                                    xp_h[li, d]
                                    .ap()
                                    .rearrange("r (mc b) -> r mc b", b=B)[ds(trow, P)],
                                )

                                ps_zr = psp.tile([P, 4, B], F32, tag=f"pszr{d}")
                                ps_h = psp.tile([P, 2, B], F32, tag=f"psh{d}")
                                # inject xp for the z,r gates via identity matmul
                                # (one whole-tile matmul opens the psum group)
                                nc.tensor.matmul(
                                    ps_zr,
                                    ident_sb,
                                    xz,
                                    start=True,
                                    stop=False,
                                )
                                for mc in range(4):
                                    for ko in range(KH):
                                        nc.tensor.matmul(
                                            ps_zr[:, mc, :],
                                            U_sb[li, d][:, ko, mc, :],
                                            hT[:, ko, :],
                                            start=False,
                                            stop=(mc == 3 and ko == KH - 1),
                                        )
                                for mc in range(2):
                                    for ko in range(KH):
                                        nc.tensor.matmul(
                                            ps_h[:, mc, :],
                                            U_sb[li, d][:, ko, 4 + mc, :],
                                            hT[:, ko, :],
                                            start=(ko == 0),
                                            stop=(ko == KH - 1),
                                        )

                                g = gtp.tile([P, 4, B], F32, tag=f"g{d}")
                                nc.scalar.activation(g, ps_zr, AF.Sigmoid)

                                tt = gtp.tile([P, 2, B], F32, tag=f"t{d}")
                                for mc in range(2):
                                    nc.vector.scalar_tensor_tensor(
                                        tt[:, mc, :],
                                        ps_h[:, mc, :],
                                        b1h_sb[li, d][:, mc : mc + 1],
                                        g[:, 2 + mc, :],
                                        op0=ALU.add,
                                        op1=ALU.mult,
                                    )
                                uu = gtp.tile([P, 2, B], F32, tag=f"u{d}")
                                nc.vector.tensor_tensor(uu, tt, xh, ALU.add)

                                cc = gtp.tile([P, 2, B], F32, tag=f"c{d}")
                                nc.scalar.activation(cc, uu, AF.Tanh)

                                dd = gtp.tile([P, 2, B], F32, tag=f"dd{d}")
                                nc.vector.tensor_tensor(dd, hm, cc, ALU.subtract)
                                ee = gtp.tile([P, 2, B], F32, tag=f"ee{d}")
                                nc.vector.tensor_tensor(ee, dd, g[:, 0:2, :], ALU.mult)
                                # bf16 shadow feeds the next matmul (critical path);
                                # fp32 master updates on gpsimd off the critical path
                                nc.vector.tensor_tensor(hT, ee, cc, ALU.add)
                                nc.gpsimd.tensor_tensor(hm, ee, cc, ALU.add)

                                col0 = 0 if d == "f" else 2
                                nc.sync.dma_start(
                                    outT.ap().rearrange("r (ko b) -> r ko b", b=B)[
                                        ds(srow, P), col0 : col0 + 2, :
                                    ],
                                    hT,
                                )

            # ============ projection phase ============
            def proj_phase():
                with (
                    tc.tile_pool(name="prps", bufs=4, space="PSUM") as psp,
                    tc.tile_pool(name="prld", bufs=3) as ldp,
                    tc.tile_pool(name="pro", bufs=4) as osp,
                ):
                    for n_i in range(NCHUNKS):
                        t0 = n_i * TCNT
                        rhs = ldp.tile([P, K2, NCHU], BF16, tag="rhs")
                        nc.sync.dma_start(
                            rhs,
                            p2T.ap().rearrange("(t p) (ko b) -> p ko t b", p=P, b=B)[
                                :, :, ds(t0, TCNT), :
                            ],
                        )
                        for mc in range(MP):
                            ps = psp.tile([P, NCHU], F32, tag="ps")
                            for ko in range(K2):
                                nc.tensor.matmul(
                                    ps,
                                    Wp_sb[:, ko, mc, :],
                                    rhs[:, ko, :],
                                    start=(ko == 0),
                                    stop=(ko == K2 - 1),
                                )
                            o = osp.tile([P, NCHU], F32, tag="o")
                            nc.scalar.activation(
                                o, ps, AF.Tanh, bias=bp_sb[:, mc : mc + 1]
                            )
                            nc.sync.dma_start(reconT[mc][:, ds(t0, TCNT), :], o)

            # ---------------- phase schedule ----------------
            if stop_after >= 1:
                xp_phase(1, lambda n_i, rhp: xT_sb[:, :, ds(n_i * NCHU, NCHU)], KD)
            if stop_after >= 2:
                scan_phase(1, p1T)

            def l2_rhs(n_i, rhp):
                rhs = rhp.tile([P, K2, NCHU], BF16, tag="rhs")
                nc.sync.dma_start(
                    rhs,
                    p1T.ap().rearrange("(t p) (ko b) -> p ko t b", p=P, b=B)[
                        :, :, ds(n_i * TCNT, TCNT), :
                    ],
                )
                return rhs

            if stop_after >= 3:
                xp_phase(2, l2_rhs, K2)
            if stop_after >= 4:
                scan_phase(2, p2T)
            if stop_after >= 5:
                proj_phase()
            else:
                # debug variants must still write the output tensor
                nc.sync.dma_start(
                    reconT[0][:, 0 : 64 // B, :], ident_sb[:, 0:64]
                )

    nc.compile()
    return nc


# ---------------------------------------------------------------------------
# host-side wrapper
# ---------------------------------------------------------------------------

_CACHED = {}


def _get_program(T: int, B: int, unroll: int):
    key = (T, B, unroll)
    if key not in _CACHED:
        _CACHED[key] = build_program(T, B, unroll)
    return _CACHED[key]


class Runner:
    """Cached PJRT executor for a compiled Bass program (SPMD over n cores).

    Mirrors bass2jax.run_bass_via_pjrt but holds onto the jitted function so
    repeat calls skip retracing / NEFF reload, and exposes a fast re-run path
    that recycles the donated output buffers (no host transfers).
    """

    def __init__(self, nc, n_cores: int):
        import jax
        from jax.sharding import Mesh, PartitionSpec
        from jax.experimental.shard_map import shard_map
        import concourse.mybir as _mybir
        from concourse import bass2jax

        bass2jax.install_neuronx_cc_hook()
        assert nc.dbg_addr is None or not nc.dbg_callbacks
        partition_name = (
            nc.partition_id_tensor.name if nc.partition_id_tensor else None
        )
        in_names, out_names, out_avals, zero_outs = [], [], [], []
        for alloc in nc.m.functions[0].allocations:
            if not isinstance(alloc, _mybir.MemoryLocationSet):
                continue
            name = alloc.memorylocations[0].name
            if alloc.kind == "ExternalInput":
                if name != partition_name:
                    in_names.append(name)
            elif alloc.kind == "ExternalOutput":
                shape = tuple(alloc.tensor_shape)
                dtype = _mybir.dt.np(alloc.dtype)
                out_names.append(name)
                out_avals.append(jax.core.ShapedArray(shape, dtype))
                zero_outs.append(np.zeros(shape, dtype))
        self.n_params = len(in_names)
        self.n_outs = len(out_avals)
        self.in_names = list(in_names)
        self.out_names = out_names
        self.out_avals = out_avals
        self.zero_outs = zero_outs
        self.n_cores = n_cores
        all_in_names = in_names + out_names
        if partition_name is not None:
            all_in_names.append(partition_name)

        def _body(*args):
            operands = list(args)
            if partition_name is not None:
                operands.append(bass2jax.partition_id_tensor())
            outs = bass2jax._bass_exec_p.bind(
                *operands,
                out_avals=tuple(out_avals),
                in_names=tuple(all_in_names),
                out_names=tuple(out_names),
                lowering_input_output_aliases=(),
                sim_require_finite=True,
                sim_require_nnan=True,
                nc=nc,
            )
            return tuple(outs)

        donate = tuple(range(self.n_params, self.n_params + self.n_outs))
        devices = jax.devices()[:n_cores]
        self.mesh = Mesh(np.asarray(devices), ("core",))
        in_specs = (PartitionSpec("core"),) * (self.n_params + self.n_outs)
        out_specs = (PartitionSpec("core"),) * self.n_outs
        self._fn = jax.jit(
            shard_map(
                _body,
                mesh=self.mesh,
                in_specs=in_specs,
                out_specs=out_specs,
                check_rep=False,
            ),
            donate_argnums=donate,
            keep_unused=True,
        )
        self._dev_in = None

    def set_inputs(self, in_maps):
        """Upload per-core inputs once (kept on device)."""
        import jax

        per_core = [
            [np.asarray(m[name]) for name in self.in_names] for m in in_maps
        ]
        concat_in = [
            np.concatenate([per_core[c][i] for c in range(self.n_cores)], axis=0)
            for i in range(self.n_params)
        ]
        self._dev_in = jax.block_until_ready(
            [jax.device_put(a) for a in concat_in]
        )

    def run(self):
        """Execute once; returns the raw (global) output arrays, blocked."""
        import jax

        zeros = [
            np.zeros((self.n_cores * z.shape[0], *z.shape[1:]), z.dtype)
            for z in self.zero_outs
        ]
        out = self._fn(*self._dev_in, *zeros)
        return jax.block_until_ready(out)

    def run_recycle(self, prev_out):
        """Re-run donating the previous outputs (zero host transfer)."""
        import jax

        out = self._fn(*self._dev_in, *prev_out)
        return jax.block_until_ready(out)

    def to_results(self, out_arrs):
        return [
            {
                name: np.asarray(out_arrs[i]).reshape(
                    self.n_cores, *self.out_avals[i].shape
                )[c]
                for i, name in enumerate(self.out_names)
            }
            for c in range(self.n_cores)
        ]


_RUNNER = {}


def _get_runner(T: int, B: int, unroll: int):
    key = (T, B, unroll)
    if key not in _RUNNER:
        _RUNNER[key] = Runner(_get_program(T, B, unroll), NCORES)
    return _RUNNER[key]


def _bf16(a):
    return np.ascontiguousarray(np.asarray(a, dtype=np.float32)).astype(
        ml_dtypes.bfloat16
    )


def _f32(a):
    return np.ascontiguousarray(np.asarray(a, dtype=np.float32))


def make_in_maps(
    encoder_hidden, decoder_input,
    W1f, U1f, b1f, W1b, U1b, b1b,
    W2f, U2f, b2f, W2b, U2b, b2b,
    Wp, bp, B_l: int,
):
    """Build the per-core input maps (host-side sharding + transposes)."""
    Bfull, T, _ = decoder_input.shape
    ncores = Bfull // B_l
    shared = {"ident": np.eye(P, dtype=np.float32)}
    for name, W, U, b in (
        ("1f", W1f, U1f, b1f), ("1b", W1b, U1b, b1b),
        ("2f", W2f, U2f, b2f), ("2b", W2b, U2b, b2b),
    ):
        b = np.asarray(b, dtype=np.float32)
        shared[f"W{name}"] = _bf16(W)
        shared[f"U{name}"] = _bf16(U)
        shared[f"bzr{name}"] = _f32(b[0, : 2 * H] + b[1, : 2 * H])
        shared[f"b0h{name}"] = _f32(b[0, 2 * H :])
        shared[f"b1h{name}"] = _f32(b[1, 2 * H :])
    shared["Wp"] = _bf16(Wp)
    shared["bp"] = _f32(bp)

    in_maps = []
    for c in range(ncores):
        xs = np.asarray(decoder_input[c * B_l : (c + 1) * B_l], dtype=np.float32)
        # [B_l, T, D] -> [D, T, B_l] -> [KD, P, T*B_l]
        xTc = xs.transpose(2, 1, 0).reshape(KD, P, T * B_l)
        hs = np.asarray(encoder_hidden[c * B_l : (c + 1) * B_l], dtype=np.float32)
        h0Tc = hs.T.reshape(KH, P, B_l)
        m = dict(shared)
        m["xT"] = xTc.astype(ml_dtypes.bfloat16)
        m["h0T"] = h0Tc.astype(np.float32)
        in_maps.append(m)
    return in_maps


def assemble_output(results, T: int, B_l: int):
    """results: list (per core) of dicts with 'reconT' [MP, P, T, B_l]."""
    ncores = len(results)
    out = np.empty((ncores * B_l, T, 256), dtype=np.float32)
    for c in range(ncores):
        rc = np.asarray(results[c]["reconT"], dtype=np.float32).reshape(256, T, B_l)
        # recon[b, t, f] = reconT[f, T-1-t, b]
        out[c * B_l : (c + 1) * B_l] = rc[:, ::-1, :].transpose(2, 1, 0)
    return out


def kernel(**inputs) -> np.ndarray:
    T = 1024
    B_l = 4
    runner = _get_runner(T, B_l, DEFAULT_UNROLL)
    in_maps = make_in_maps(**inputs, B_l=B_l)
    runner.set_inputs(in_maps)
    out = runner.run()
    return assemble_output(runner.to_results(out), T, B_l)


if __name__ == "__main__":
    # quick shape smoke test of the host-side prep
    rng = np.random.default_rng(0)
    print("building program (T=16 smoke)...")
    build_program(16, 4, 4)
    print("ok")



# revision 2
# speedup vs baseline: 13.0016x; 13.0016x over previous
"""Trainium2 Bass kernel for a 2-layer bidirectional GRU decoder.

Problem (hardcoded shapes): B=32, T=1024, D=256, H=256, P=256.
  p1 = bidir_gru(decoder_input, h0, W1f/U1f, W1b/U1b)        # [B,T,512]
  p2 = bidir_gru(p1,            h0, W2f/U2f, W2b/U2b)        # [B,T,512]
  recon = tanh(p2[:, ::-1, :] @ Wp + bp)                     # [B,T,256]

Strategy: pure data parallel over batch (32 = 8 cores x 4). Each core runs
all four GRU scans for its batch shard; no cross-core traffic. Everything is
kept in a transposed layout [feature -> partitions, batch -> free] so that
the recurrent matmul streams h (tiny) against stationary U chunks (bf16,
fast-weight-load), gate pre-activations are injected into PSUM via an
identity matmul, and all per-feature biases fold into per-partition
activation bias operands.
"""

import sys

if "/opt/trn_rl_repo" not in sys.path:
    sys.path.insert(0, "/opt/trn_rl_repo")

import numpy as np
import ml_dtypes

import concourse.bass as bass
import concourse.bacc as bacc
import concourse.mybir as mybir
import concourse.tile as tile
from concourse.bass import ds, ts

BF16 = mybir.dt.bfloat16
F32 = mybir.dt.float32
AF = mybir.ActivationFunctionType
ALU = mybir.AluOpType

P = 128
D = 256
H = 256
H3 = 3 * H
KD = D // P            # 2  k-chunks of the layer-1 input dim
KH = H // P            # 2  k-chunks of the hidden dim
K2 = (2 * H) // P      # 4  k-chunks of the layer-2 input dim
M3 = H3 // P           # 6  m-chunks of the 3H gate dim
MP = 256 // P          # 2  m-chunks of the projection output
NCORES = 8
DIRS = ("f", "b")
DEFAULT_UNROLL = 8


def build_program(T: int, B: int, unroll: int, stop_after: int = 5):
    """Build the per-core SPMD program. Returns (nc, meta).

    stop_after: debug knob — 1=xp1, 2=+scan1, 3=+xp2, 4=+scan2, 5=full.
    """
    assert T % unroll == 0
    NT = T * B
    NCHU = min(512, NT)            # matmul moving-dim chunk
    assert NT % NCHU == 0
    NCHUNKS = NT // NCHU
    TCNT = NCHU // B               # timesteps covered per chunk

    nc = bacc.Bacc("TRN2", num_devices=NCORES, debug=False)

    # ---------------- DRAM I/O ----------------
    xT = nc.dram_tensor("xT", [KD, P, NT], BF16, kind="ExternalInput")
    h0T = nc.dram_tensor("h0T", [KH, P, B], F32, kind="ExternalInput")
    ident = nc.dram_tensor("ident", [P, P], F32, kind="ExternalInput")

    Wd, Ud, bzr, b0h, b1h = {}, {}, {}, {}, {}
    for li, kin in ((1, D), (2, 2 * H)):
        for d in DIRS:
            Wd[li, d] = nc.dram_tensor(f"W{li}{d}", [kin, H3], BF16, kind="ExternalInput")
            Ud[li, d] = nc.dram_tensor(f"U{li}{d}", [H, H3], BF16, kind="ExternalInput")
            bzr[li, d] = nc.dram_tensor(f"bzr{li}{d}", [2 * H], F32, kind="ExternalInput")
            b0h[li, d] = nc.dram_tensor(f"b0h{li}{d}", [H], F32, kind="ExternalInput")
            b1h[li, d] = nc.dram_tensor(f"b1h{li}{d}", [H], F32, kind="ExternalInput")
    Wp = nc.dram_tensor("Wp", [2 * H, 256], BF16, kind="ExternalInput")
    bp = nc.dram_tensor("bp", [256], F32, kind="ExternalInput")

    # scratch (per-core DRAM)
    xp_zr, xp_h = {}, {}
    for li in (1, 2):
        for d in DIRS:
            xp_zr[li, d] = nc.dram_tensor(f"xpzr{li}{d}", [T * P, 4 * B], F32, kind="Internal")
            xp_h[li, d] = nc.dram_tensor(f"xph{li}{d}", [T * P, 2 * B], F32, kind="Internal")
    p1T = nc.dram_tensor("p1T", [T * P, 4 * B], BF16, kind="Internal")
    p2T = nc.dram_tensor("p2T", [T * P, 4 * B], BF16, kind="Internal")

    reconT = nc.dram_tensor("reconT", [MP, P, T, B], F32, kind="ExternalOutput")

    with tile.TileContext(nc) as tc:
        with tc.tile_pool(name="persist", bufs=1) as pers:
            # ---- load persistent SBUF data ----
            xT_sb = pers.tile([P, KD, NT], BF16)
            for ko in range(KD):
                nc.sync.dma_start(xT_sb[:, ko, :], xT[ko])

            ident_sb = pers.tile([P, P], F32)
            nc.sync.dma_start(ident_sb, ident[:])

            h0_sb = pers.tile([P, KH, B], F32)
            nc.sync.dma_start(h0_sb, h0T.ap().rearrange("ko p b -> p ko b"))

            W_sb, U_sb, bzr_sb, b0h_sb, b1h_sb = {}, {}, {}, {}, {}
            for li, kin in ((1, D), (2, 2 * H)):
                for d in DIRS:
                    kck = kin // P
                    W_sb[li, d] = pers.tile([P, kck, M3, P], BF16, name=f"W{li}{d}_sb")
                    nc.sync.dma_start(
                        W_sb[li, d],
                        Wd[li, d].ap().rearrange("(ko p) (mc q) -> p ko mc q", p=P, q=P),
                    )
                    U_sb[li, d] = pers.tile([P, KH, M3, P], BF16, name=f"U{li}{d}_sb")
                    nc.sync.dma_start(
                        U_sb[li, d],
                        Ud[li, d].ap().rearrange("(ko p) (mc q) -> p ko mc q", p=P, q=P),
                    )
                    bzr_sb[li, d] = pers.tile([P, 4], F32, name=f"bzr{li}{d}_sb")
                    nc.sync.dma_start(
                        bzr_sb[li, d], bzr[li, d].ap().rearrange("(mc p) -> p mc", p=P)
                    )
                    b0h_sb[li, d] = pers.tile([P, 2], F32, name=f"b0h{li}{d}_sb")
                    nc.sync.dma_start(
                        b0h_sb[li, d], b0h[li, d].ap().rearrange("(mc p) -> p mc", p=P)
                    )
                    b1h_sb[li, d] = pers.tile([P, 2], F32, name=f"b1h{li}{d}_sb")
                    nc.sync.dma_start(
                        b1h_sb[li, d], b1h[li, d].ap().rearrange("(mc p) -> p mc", p=P)
                    )
            Wp_sb = pers.tile([P, K2, MP, P], BF16)
            nc.sync.dma_start(
                Wp_sb, Wp.ap().rearrange("(ko p) (mc q) -> p ko mc q", p=P, q=P)
            )
            bp_sb = pers.tile([P, MP], F32)
            nc.sync.dma_start(bp_sb, bp.ap().rearrange("(mc p) -> p mc", p=P))

            # GRU hidden state tiles (persist across one scan phase):
            # fp32 master (updated on gpsimd) + bf16 shadow fed to the PE
            h32 = {d: pers.tile([P, KH, B], F32, name=f"h32_{d}") for d in DIRS}
            hbf = {d: pers.tile([P, KH, B], BF16, name=f"hbf_{d}") for d in DIRS}

            # ============ input-projection phase ============
            def xp_phase(li: int, rhs_src, kck: int):
                """xp[li,d] = rhs @ W[li,d] + biases, written as per-step records.

                rhs_src(n_i, pool) -> SBUF AP [P, kck, NCHU] for columns of (t, b).
                """
                with (
                    tc.tile_pool(name=f"xpps{li}", bufs=4, space="PSUM") as psp,
                    tc.tile_pool(name=f"xpo{li}", bufs=4) as osp,
                    tc.tile_pool(name=f"xprhs{li}", bufs=3) as rhp,
                ):
                    for n_i in range(NCHUNKS):
                        rhs = rhs_src(n_i, rhp)
                        t0 = n_i * TCNT
                        for d in DIRS:
                            for mc in range(M3):
                                ps = psp.tile([P, NCHU], F32, tag="ps")
                                for ko in range(kck):
                                    nc.tensor.matmul(
                                        ps,
                                        W_sb[li, d][:, ko, mc, :],
                                        rhs[:, ko, :],
                                        start=(ko == 0),
                                        stop=(ko == kck - 1),
                                    )
                                if mc < 4:
                                    o = osp.tile([P, NCHU], F32, tag="ozr")
                                    dst = (
                                        xp_zr[li, d]
                                        .ap()
                                        .rearrange("(t p) (mc b) -> p mc t b", p=P, b=B)
                                    )[:, mc, ds(t0, TCNT), :]
                                else:
                                    o = osp.tile([P, NCHU], F32, tag="oh")
                                    dst = (
                                        xp_h[li, d]
                                        .ap()
                                        .rearrange("(t p) (mc b) -> p mc t b", p=P, b=B)
                                    )[:, mc - 4, ds(t0, TCNT), :]
                                if mc % 2 == 0:
                                    bias_ap = (
                                        bzr_sb[li, d][:, mc : mc + 1]
                                        if mc < 4
                                        else b0h_sb[li, d][:, mc - 4 : mc - 3]
                                    )
                                    nc.scalar.activation(o, ps, AF.Identity, bias=bias_ap)
                                else:
                                    bias_ap = (
                                        bzr_sb[li, d][:, mc : mc + 1]
                                        if mc < 4
                                        else b0h_sb[li, d][:, mc - 4 : mc - 3]
                                    )
                                    nc.vector.tensor_tensor(
                                        o, ps, bias_ap.to_broadcast((P, NCHU)), ALU.add
                                    )
                                nc.sync.dma_start(dst, o)

            # ============ recurrent scan phase ============
            def scan_phase(li: int, outT):
                # reset hidden state to h0
                for d in DIRS:
                    nc.vector.tensor_copy(h32[d], h0_sb)
                    nc.vector.tensor_copy(hbf[d], h0_sb)
                with (
                    tc.tile_pool(name=f"ld{li}", bufs=4) as ldp,
                    tc.tile_pool(name=f"ps{li}", bufs=2, space="PSUM") as psp,
                    tc.tile_pool(name=f"gt{li}", bufs=2) as gtp,
                ):
                    with tc.For_i(0, T, unroll) as i0:
                        for u in range(unroll):
                            for d in DIRS:
                                hT = hbf[d]
                                hm = h32[d]
                                if d == "f":
                                    trow = (i0 + u) * P
                                else:
                                    trow = ((T - 1 - u) - i0) * P
                                srow = (i0 + u) * P  # storage row (scan index)

                                xz = ldp.tile([P, 4, B], F32, tag=f"xz{d}")
                                nc.sync.dma_start(
                                    xz,
                                    xp_zr[li, d]
                                    .ap()
                                    .rearrange("r (mc b) -> r mc b", b=B)[ds(trow, P)],
                                )
                                xh = ldp.tile([P, 2, B], F32, tag=f"xh{d}")
                                nc.sync.dma_start(
                                    xh,
                                    xp_h[li, d]
                                    .ap()
                                    .rearrange("r (mc b) -> r mc b", b=B)[ds(trow, P)],
                                )

                                ps_zr = psp.tile([P, 4, B], F32, tag=f"pszr{d}")
                                ps_h = psp.tile([P, 2, B], F32, tag=f"psh{d}")
                                # inject xp for the z,r gates via identity matmul
                                # (one whole-tile matmul opens the psum group)
                                nc.tensor.matmul(
                                    ps_zr,
                                    ident_sb,
                                    xz,
                                    start=True,
                                    stop=False,
                                )
                                for mc in range(4):
                                    for ko in range(KH):
                                        nc.tensor.matmul(
                                            ps_zr[:, mc, :],
                                            U_sb[li, d][:, ko, mc, :],
                                            hT[:, ko, :],
                                            start=False,
                                            stop=(mc == 3 and ko == KH - 1),
                                        )
                                for mc in range(2):
                                    for ko in range(KH):
                                        nc.tensor.matmul(
                                            ps_h[:, mc, :],
                                            U_sb[li, d][:, ko, 4 + mc, :],
                                            hT[:, ko, :],
                                            start=(ko == 0),
                                            stop=(ko == KH - 1),
                                        )

                                g = gtp.tile([P, 4, B], F32, tag=f"g{d}")
                                nc.scalar.activation(g, ps_zr, AF.Sigmoid)

                                tt = gtp.tile([P, 2, B], F32, tag=f"t{d}")
                                for mc in range(2):
                                    nc.vector.scalar_tensor_tensor(
                                        tt[:, mc, :],
                                        ps_h[:, mc, :],
                                        b1h_sb[li, d][:, mc : mc + 1],
                                        g[:, 2 + mc, :],
                                        op0=ALU.add,
                                        op1=ALU.mult,
                                    )
                                uu = gtp.tile([P, 2, B], F32, tag=f"u{d}")
                                nc.vector.tensor_tensor(uu, tt, xh, ALU.add)

                                cc = gtp.tile([P, 2, B], F32, tag=f"c{d}")
                                nc.scalar.activation(cc, uu, AF.Tanh)

                                dd = gtp.tile([P, 2, B], F32, tag=f"dd{d}")
                                nc.vector.tensor_tensor(dd, hm, cc, ALU.subtract)
                                ee = gtp.tile([P, 2, B], F32, tag=f"ee{d}")
                                nc.vector.tensor_tensor(ee, dd, g[:, 0:2, :], ALU.mult)
                                # bf16 shadow feeds the next matmul (critical path);
                                # fp32 master updates on gpsimd off the critical path
                                nc.vector.tensor_tensor(hT, ee, cc, ALU.add)
                                nc.gpsimd.tensor_tensor(hm, ee, cc, ALU.add)

                                col0 = 0 if d == "f" else 2
                                nc.sync.dma_start(
                                    outT.ap().rearrange("r (ko b) -> r ko b", b=B)[
                                        ds(srow, P), col0 : col0 + 2, :
                                    ],
                                    hT,
                                )

            # ============ projection phase ============
            def proj_phase():
                with (
                    tc.tile_pool(name="prps", bufs=4, space="PSUM") as psp,
                    tc.tile_pool(name="prld", bufs=3) as ldp,
                    tc.tile_pool(name="pro", bufs=4) as osp,
                ):
                    for n_i in range(NCHUNKS):
                        t0 = n_i * TCNT
                        rhs = ldp.tile([P, K2, NCHU], BF16, tag="rhs")
                        nc.sync.dma_start(
                            rhs,
                            p2T.ap().rearrange("(t p) (ko b) -> p ko t b", p=P, b=B)[
                                :, :, ds(t0, TCNT), :
                            ],
                        )
                        for mc in range(MP):
                            ps = psp.tile([P, NCHU], F32, tag="ps")
                            for ko in range(K2):
                                nc.tensor.matmul(
                                    ps,
                                    Wp_sb[:, ko, mc, :],
                                    rhs[:, ko, :],
                                    start=(ko == 0),
                                    stop=(ko == K2 - 1),
                                )
                            o = osp.tile([P, NCHU], F32, tag="o")
                            nc.scalar.activation(
                                o, ps, AF.Tanh, bias=bp_sb[:, mc : mc + 1]
                            )
                            nc.sync.dma_start(reconT[mc][:, ds(t0, TCNT), :], o)

            # ---------------- phase schedule ----------------
            if stop_after >= 1:
                xp_phase(1, lambda n_i, rhp: xT_sb[:, :, ds(n_i * NCHU, NCHU)], KD)
            if stop_after >= 2:
                scan_phase(1, p1T)

            def l2_rhs(n_i, rhp):
                rhs = rhp.tile([P, K2, NCHU], BF16, tag="rhs")
                nc.sync.dma_start(
                    rhs,
                    p1T.ap().rearrange("(t p) (ko b) -> p ko t b", p=P, b=B)[
                        :, :, ds(n_i * TCNT, TCNT), :
                    ],
                )
                return rhs

            if stop_after >= 3:
                xp_phase(2, l2_rhs, K2)
            if stop_after >= 4:
                scan_phase(2, p2T)
            if stop_after >= 5:
                proj_phase()
            else:
                # debug variants must still write the output tensor
                nc.sync.dma_start(
                    reconT[0][:, 0 : 64 // B, :], ident_sb[:, 0:64]
                )

    nc.compile()
    return nc


# ---------------------------------------------------------------------------
# host-side wrapper
# ---------------------------------------------------------------------------

_CACHED = {}


def _get_program(T: int, B: int, unroll: int):
    key = (T, B, unroll)
    if key not in _CACHED:
        _CACHED[key] = build_program(T, B, unroll)
    return _CACHED[key]


class Runner:
    """Cached PJRT executor for a compiled Bass program (SPMD over n cores).

    Mirrors bass2jax.run_bass_via_pjrt but holds onto the jitted function so
    repeat calls skip retracing / NEFF reload, and exposes a fast re-run path
    that recycles the donated output buffers (no host transfers).
    """

    def __init__(self, nc, n_cores: int):
        import jax
        from jax.sharding import Mesh, PartitionSpec
        from jax.experimental.shard_map import shard_map
        import concourse.mybir as _mybir
        from concourse import bass2jax

        bass2jax.install_neuronx_cc_hook()
        assert nc.dbg_addr is None or not nc.dbg_callbacks
        partition_name = (
            nc.partition_id_tensor.name if nc.partition_id_tensor else None
        )
        in_names, out_names, out_avals, zero_outs = [], [], [], []
        for alloc in nc.m.functions[0].allocations:
            if not isinstance(alloc, _mybir.MemoryLocationSet):
                continue
            name = alloc.memorylocations[0].name
            if alloc.kind == "ExternalInput":
                if name != partition_name:
                    in_names.append(name)
            elif alloc.kind == "ExternalOutput":
                shape = tuple(alloc.tensor_shape)
                dtype = _mybir.dt.np(alloc.dtype)
                out_names.append(name)
                out_avals.append(jax.core.ShapedArray(shape, dtype))
                zero_outs.append(np.zeros(shape, dtype))
        self.n_params = len(in_names)
        self.n_outs = len(out_avals)
        self.in_names = list(in_names)
        self.out_names = out_names
        self.out_avals = out_avals
        self.zero_outs = zero_outs
        self.n_cores = n_cores
        all_in_names = in_names + out_names
        if partition_name is not None:
            all_in_names.append(partition_name)

        def _body(*args):
            operands = list(args)
            if partition_name is not None:
                operands.append(bass2jax.partition_id_tensor())
            outs = bass2jax._bass_exec_p.bind(
                *operands,
                out_avals=tuple(out_avals),
                in_names=tuple(all_in_names),
                out_names=tuple(out_names),
                lowering_input_output_aliases=(),
                sim_require_finite=True,
                sim_require_nnan=True,
                nc=nc,
            )
            return tuple(outs)

        donate = tuple(range(self.n_params, self.n_params + self.n_outs))
        devices = jax.devices()[:n_cores]
        self.mesh = Mesh(np.asarray(devices), ("core",))
        in_specs = (PartitionSpec("core"),) * (self.n_params + self.n_outs)
        out_specs = (PartitionSpec("core"),) * self.n_outs
        self._fn = jax.jit(
            shard_map(
                _body,
                mesh=self.mesh,
                in_specs=in_specs,
                out_specs=out_specs,
                check_rep=False,
            ),
            donate_argnums=donate,
            keep_unused=True,
        )
        self._dev_in = None

    def set_inputs(self, in_maps):
        """Upload per-core inputs once (kept on device, pre-sharded)."""
        import jax
        from jax.sharding import NamedSharding, PartitionSpec

        sharding = NamedSharding(self.mesh, PartitionSpec("core"))
        per_core = [
            [np.asarray(m[name]) for name in self.in_names] for m in in_maps
        ]
        concat_in = [
            np.concatenate([per_core[c][i] for c in range(self.n_cores)], axis=0)
            for i in range(self.n_params)
        ]
        self._dev_in = jax.block_until_ready(
            [jax.device_put(a, sharding) for a in concat_in]
        )

    def run(self):
        """Execute once; returns the raw (global) output arrays, blocked."""
        import jax

        zeros = [
            np.zeros((self.n_cores * z.shape[0], *z.shape[1:]), z.dtype)
            for z in self.zero_outs
        ]
        out = self._fn(*self._dev_in, *zeros)
        return jax.block_until_ready(out)

    def run_recycle(self, prev_out):
        """Re-run donating the previous outputs (zero host transfer)."""
        import jax

        out = self._fn(*self._dev_in, *prev_out)
        return jax.block_until_ready(out)

    def to_results(self, out_arrs):
        return [
            {
                name: np.asarray(out_arrs[i]).reshape(
                    self.n_cores, *self.out_avals[i].shape
                )[c]
                for i, name in enumerate(self.out_names)
            }
            for c in range(self.n_cores)
        ]


_RUNNER = {}


def _get_runner(T: int, B: int, unroll: int):
    key = (T, B, unroll)
    if key not in _RUNNER:
        _RUNNER[key] = Runner(_get_program(T, B, unroll), NCORES)
    return _RUNNER[key]


def _bf16(a):
    return np.ascontiguousarray(np.asarray(a, dtype=np.float32)).astype(
        ml_dtypes.bfloat16
    )


def _f32(a):
    return np.ascontiguousarray(np.asarray(a, dtype=np.float32))


def make_in_maps(
    encoder_hidden, decoder_input,
    W1f, U1f, b1f, W1b, U1b, b1b,
    W2f, U2f, b2f, W2b, U2b, b2b,
    Wp, bp, B_l: int,
):
    """Build the per-core input maps (host-side sharding + transposes)."""
    Bfull, T, _ = decoder_input.shape
    ncores = Bfull // B_l
    shared = {"ident": np.eye(P, dtype=np.float32)}
    for name, W, U, b in (
        ("1f", W1f, U1f, b1f), ("1b", W1b, U1b, b1b),
        ("2f", W2f, U2f, b2f), ("2b", W2b, U2b, b2b),
    ):
        b = np.asarray(b, dtype=np.float32)
        shared[f"W{name}"] = _bf16(W)
        shared[f"U{name}"] = _bf16(U)
        shared[f"bzr{name}"] = _f32(b[0, : 2 * H] + b[1, : 2 * H])
        shared[f"b0h{name}"] = _f32(b[0, 2 * H :])
        shared[f"b1h{name}"] = _f32(b[1, 2 * H :])
    shared["Wp"] = _bf16(Wp)
    shared["bp"] = _f32(bp)

    in_maps = []
    for c in range(ncores):
        xs = np.asarray(decoder_input[c * B_l : (c + 1) * B_l], dtype=np.float32)
        # [B_l, T, D] -> [D, T, B_l] -> [KD, P, T*B_l]
        xTc = xs.transpose(2, 1, 0).reshape(KD, P, T * B_l)
        hs = np.asarray(encoder_hidden[c * B_l : (c + 1) * B_l], dtype=np.float32)
        h0Tc = hs.T.reshape(KH, P, B_l)
        m = dict(shared)
        m["xT"] = xTc.astype(ml_dtypes.bfloat16)
        m["h0T"] = h0Tc.astype(np.float32)
        in_maps.append(m)
    return in_maps


def assemble_output(results, T: int, B_l: int):
    """results: list (per core) of dicts with 'reconT' [MP, P, T, B_l]."""
    ncores = len(results)
    out = np.empty((ncores * B_l, T, 256), dtype=np.float32)
    for c in range(ncores):
        rc = np.asarray(results[c]["reconT"], dtype=np.float32).reshape(256, T, B_l)
        # recon[b, t, f] = reconT[f, T-1-t, b]
        out[c * B_l : (c + 1) * B_l] = rc[:, ::-1, :].transpose(2, 1, 0)
    return out


def kernel(**inputs) -> np.ndarray:
    T = 1024
    B_l = 4
    runner = _get_runner(T, B_l, DEFAULT_UNROLL)
    in_maps = make_in_maps(**inputs, B_l=B_l)
    runner.set_inputs(in_maps)
    out = runner.run()
    return assemble_output(runner.to_results(out), T, B_l)


if __name__ == "__main__":
    # quick shape smoke test of the host-side prep
    rng = np.random.default_rng(0)
    print("building program (T=16 smoke)...")
    build_program(16, 4, 4)
    print("ok")

